# revision 1
# baseline (speedup 1.0000x reference)
"""Trainium2 Bass kernel for the ConvolutionalKAN problem.

Math: the KAN conv
    out[b,o,y,x] = sum_{j,kk,l,m} phi_m(11*inp[b,j,y+kk,x+l]) * coeff[o,j,kk,l,m]
with phi_m the degree-3 B-spline basis on uniform knots linspace(0,1,12).
Since the knots are uniform, phi_m(t) = N3(t-m) with N3 the cardinal cubic
B-spline, which has the branch-free closed form (u = |t-(m+2)|):
    6*N3 = relu(2-u)^3 - (4^(1/3)*relu(1-u))^3
All 8 basis channels are bounded by 4 (vs 11^3 for the naive truncated-power
form), so the float32r (TF32-like ~12-bit) matmul path is accurate to ~1e-4
while streaming at full PE rate. Weights fold to coeff/6 exactly, making
this a VALID 3x3 conv over 64*8 = 512 input channels.

Sharding: data-parallel over batch, 2 images per core on 8 cores.
On-chip: per 18-row strip, the 8 basis channels (4 tiles of 128 partitions,
2 basis fns x 64 input channels each) are computed on ScalarE (Abs/Relu/
Square with per-partition bias tables) + VectorE (cube muls) + GpSimdE,
then consumed by TensorE as chains of 36 accumulating float32r matmuls per
8-row output group (4 c-tiles x 9 taps, N = rows*62 <= 496, one PSUM bank).
"""

import os
import sys

import numpy as np

for _p in ("/root/.axon_site/_ro/trn_rl_repo", "/opt/trn_rl_repo"):
    if os.path.isdir(_p) and _p not in sys.path:
        sys.path.append(_p)

B_FULL = 16
N_CORES = 8
B_SHARD = B_FULL // N_CORES
CIN = 64
COUT = 64
H = 64
W = 64
KS = 3
NB = 8
NS = 8  # B-spline basis channels per input channel
HO = H - KS + 1  # 62
WO = W - KS + 1  # 62
NQ = (CIN * NS) // 128  # 6 contraction tiles of 128
TAPS = KS * KS

# output row strips: (y0, n_input_rows, per-group output rows)
STRIPS = [(0, 18, (8, 8)), (16, 18, (8, 8)), (32, 18, (8, 8)), (48, 16, (8, 6))]

MM_DTYPE_ENV = os.environ.get("KAN_MM_DTYPE", "float32r")
C2 = 4.0 ** (1.0 / 3.0)


SPLIT = 5.5  # value-domain split point for the dual-rail clamped basis


def _fold_coeff(coeff: np.ndarray):
    """coeff [COUT, CIN, KS, KS, NB] -> (W_host [512, 576] f32, obias [128,1]).

    Channels fed to the matmul are 6*phi_m(t) = relu(2-u)^3 - (c2*relu(1-u))^3
    with u = |t-(m+2)|, so the folded weights are just coeff/6 in layout
    W_host[m*64 + j, (kk*3+l)*64 + o].
    """
    w = (coeff.astype(np.float64).transpose(4, 1, 2, 3, 0) / 6.0).reshape(
        NS * CIN, TAPS * COUT)
    return (np.ascontiguousarray(w, dtype=np.float32),
            np.zeros((128, 1), dtype=np.float32))


def _build_bass():
    import concourse.bacc as bacc
    import concourse.mybir as mybir
    import concourse.tile as tile

    f32 = mybir.dt.float32
    mm_dt = getattr(mybir.dt, MM_DTYPE_ENV)
    use_f32r = mm_dt != f32
    AF = mybir.ActivationFunctionType

    nc = bacc.Bacc("TRN2", target_bir_lowering=False, debug=False,
                   num_devices=N_CORES)
    x_d = nc.dram_tensor("x", [B_SHARD, CIN, H, W], f32, kind="ExternalInput").ap()
    w_d = nc.dram_tensor("w", [NS * CIN, TAPS * COUT], f32, kind="ExternalInput").ap()
    b_d = nc.dram_tensor("btbl", [128, NQ + 2], f32, kind="ExternalInput").ap()
    ob_d = nc.dram_tensor("obias", [128, 1], f32, kind="ExternalInput").ap()
    out_d = nc.dram_tensor("out", [B_SHARD, COUT, HO, WO], f32,
                           kind="ExternalOutput").ap()

    col_tile = os.environ.get("KAN_COLTILE", "0") == "1"
    wsplit = os.environ.get("KAN_WSPLIT", "0") == "1"
    w_dt = f32 if os.environ.get("KAN_W_EXACT", "0") == "1" else mm_dt
    if os.environ.get("KAN_W_BF16", "0") == "1":
        w_dt = mybir.dt.bfloat16

    with tile.TileContext(nc) as tc:
        from contextlib import ExitStack

        with ExitStack() as ctx:
            wpool = ctx.enter_context(tc.tile_pool(name="w", bufs=NQ))
            cpool = ctx.enter_context(tc.tile_pool(name="const", bufs=1))
            xpool = ctx.enter_context(tc.tile_pool(name="x", bufs=3))
            gpool = ctx.enter_context(tc.tile_pool(name="g", bufs=4 * NQ))
            rpool = ctx.enter_context(tc.tile_pool(name="r", bufs=2))
            spool = ctx.enter_context(tc.tile_pool(name="sq", bufs=2))
            opool = ctx.enter_context(tc.tile_pool(name="o", bufs=4))
            ppool = ctx.enter_context(
                tc.tile_pool(name="ps", bufs=4, space="PSUM"))

            bt = cpool.tile([128, NQ + 2], f32)
            nc.sync.dma_start(bt[:], b_d[:])
            obt = cpool.tile([128, 1], f32, tag="obias")
            nc.sync.dma_start(obt[:], ob_d[:])
            wts = []
            for q in range(NQ):
                wt = wpool.tile([128, TAPS * COUT], f32, tag="wstage")
                nc.sync.dma_start(wt[:], w_d[q * 128:(q + 1) * 128, :])
                if wsplit:
                    # [w_hi | w_lo] per tap: wc[:, tap*128:+64] = f32r(w),
                    # [+64:+128] = f32r(w - w_hi); stationary M=128 costs no
                    # extra PE time and makes weights ~24-bit effective.
                    wc = wpool.tile([128, 2 * TAPS * COUT], mm_dt, tag="wc")
                    wcv = wc[:].rearrange("p (t h o) -> p t h o", h=2, o=COUT)
                    wtv = wt[:].rearrange("p (t o) -> p t o", o=COUT)
                    nc.vector.tensor_copy(wcv[:, :, 0, :], wtv)
                    nc.vector.tensor_sub(wcv[:, :, 1, :], wtv, wcv[:, :, 0, :])
                    wts.append(wc)
                elif w_dt != f32:
                    wr = wpool.tile([128, TAPS * COUT], w_dt, tag="wr")
                    nc.vector.tensor_copy(wr[:], wt[:])
                    wts.append(wr)
                else:
                    wts.append(wt)

            alu = mybir.AluOpType
            NPIX = 18 * W
            for b in range(B_SHARD):
                for (y0, nin, groups) in STRIPS:
                    npx = nin * W
                    xt = xpool.tile([128, NPIX], f32)
                    src = x_d[b, :, y0:y0 + nin, :]
                    nc.gpsimd.dma_start(
                        xt[0:64].rearrange("p (r c) -> p r c", c=W)[:, :nin, :],
                        src)
                    nc.gpsimd.dma_start(
                        xt[64:128].rearrange("p (r c) -> p r c", c=W)[:, :nin, :],
                        src)
                    gts = []
                    for q in range(NQ):
                        # u = |11x - (m+2)|; 6*phi = relu(2-u)^3 - (c2*relu(1-u))^3
                        g = gpool.tile([128, NPIX], mm_dt)
                        u = rpool.tile([128, NPIX], f32, tag="u")
                        a = rpool.tile([128, NPIX], f32, tag="a")
                        bb = rpool.tile([128, NPIX], f32, tag="b")
                        sa = spool.tile([128, NPIX], f32, tag="sa")
                        sb = spool.tile([128, NPIX], f32, tag="sb")
                        a3 = spool.tile([128, NPIX], f32, tag="a3")
                        bias = bt[:, q:q + 1]
                        nc.scalar.activation(u[:, :npx], xt[:, :npx],
                                             AF.Abs, bias=bias, scale=11.0)
                        nc.scalar.activation(a[:, :npx], u[:, :npx],
                                             AF.Relu, bias=bt[:, NQ:NQ + 1],
                                             scale=-1.0)
                        nc.scalar.activation(bb[:, :npx], u[:, :npx],
                                             AF.Relu, bias=bt[:, NQ + 1:NQ + 2],
                                             scale=-C2)
                        if q < 4:
                            nc.scalar.activation(sa[:, :npx], u[:, :npx],
                                                 AF.Square,
                                                 bias=bt[:, NQ:NQ + 1],
                                                 scale=-1.0)
                        else:
                            nc.vector.tensor_mul(sa[:, :npx], a[:, :npx],
                                                 a[:, :npx])
                        nc.gpsimd.tensor_mul(sb[:, :npx], bb[:, :npx],
                                             bb[:, :npx])
                        nc.vector.tensor_mul(a3[:, :npx], sa[:, :npx],
                                             a[:, :npx])
                        nc.vector.tensor_mul(sb[:, :npx], sb[:, :npx],
                                             bb[:, :npx])
                        nc.vector.tensor_sub(g[:, :npx], a3[:, :npx],
                                             sb[:, :npx])
                        gts.append(g)

                    gvs = [g[:].rearrange("p (r c) -> p r c", c=W) for g in gts]
                    n_mm = NQ * TAPS
                    if wsplit:
                        for grp, nr in enumerate(groups):
                            ps = ppool.tile([128, 8, WO], f32)
                            i_mm = 0
                            for q in range(NQ):
                                for kk in range(KS):
                                    for l in range(KS):
                                        r0 = 8 * grp + kk
                                        rhs = gvs[q][:, r0:r0 + nr, l:l + WO]
                                        tap = kk * KS + l
                                        lhsT = wts[q][:, tap * 2 * COUT:
                                                      (tap + 1) * 2 * COUT]
                                        nc.tensor.matmul(
                                            ps[:, :nr, :], lhsT, rhs,
                                            start=(i_mm == 0),
                                            stop=(i_mm == n_mm - 1),
                                        )
                                        i_mm += 1
                            ot = opool.tile([64, 8, WO], f32)
                            nc.scalar.activation(ot[:, :nr, :], ps[0:64, :nr, :],
                                                 AF.Identity, bias=obt[0:64, 0:1],
                                                 scale=1.0)
                            nc.vector.scalar_tensor_tensor(
                                ot[:, :nr, :], ps[64:128, :nr, :], 1.0,
                                ot[:, :nr, :], op0=alu.mult, op1=alu.add)
                            nc.sync.dma_start(
                                out_d[b, :, y0 + 8 * grp:y0 + 8 * grp + nr, :],
                                ot[:, :nr, :])
                    elif col_tile:
                        # both output groups in one chain, 2 col-tiles
                        ps = ppool.tile([128, 8, WO], f32)
                        i_mm = 0
                        for q in range(NQ):
                            for kk in range(KS):
                                for l in range(KS):
                                    lhsT = wts[q][:, (kk * KS + l) * COUT:
                                                  (kk * KS + l + 1) * COUT]
                                    for grp, nr in enumerate(groups):
                                        r0 = 8 * grp + kk
                                        rhs = gvs[q][:, r0:r0 + nr, l:l + WO]
                                        nc.tensor.matmul(
                                            ps[64 * grp:64 * grp + 64, :nr, :],
                                            lhsT, rhs,
                                            start=(i_mm == 0),
                                            stop=(i_mm == n_mm - 1),
                                            tile_position=(0, 64 * grp),
                                        )
                                    i_mm += 1
                        ot = opool.tile([128, 8, WO], f32)
                        for grp, nr in enumerate(groups):
                            sl = slice(64 * grp, 64 * grp + 64)
                            nc.scalar.activation(ot[sl, :nr, :], ps[sl, :nr, :],
                                                 AF.Identity, bias=obt[sl, 0:1],
                                                 scale=1.0)
                            nc.sync.dma_start(
                                out_d[b, :, y0 + 8 * grp:y0 + 8 * grp + nr, :],
                                ot[sl, :nr, :])
                    else:
                        for grp, nr in enumerate(groups):
                            ps = ppool.tile([64, 8, WO], f32)
                            i_mm = 0
                            for q in range(NQ):
                                for kk in range(KS):
                                    for l in range(KS):
                                        r0 = 8 * grp + kk
                                        rhs = gvs[q][:, r0:r0 + nr, l:l + WO]
                                        lhsT = wts[q][:, (kk * KS + l) * COUT:
                                                      (kk * KS + l + 1) * COUT]
                                        nc.tensor.matmul(
                                            ps[:, :nr, :], lhsT, rhs,
                                            start=(i_mm == 0),
                                            stop=(i_mm == n_mm - 1),
                                        )
                                        i_mm += 1
                            ot = opool.tile([64, 8, WO], f32)
                            nc.vector.tensor_copy(ot[:, :nr, :], ps[:, :nr, :])
                            nc.sync.dma_start(
                                out_d[b, :, y0 + 8 * grp:y0 + 8 * grp + nr, :],
                                ot[:, :nr, :])

    nc.compile()
    return nc


def _maybe_install_profile_shim():
    """Allow trace=True/BASS_TRACE under axon even though this image lacks
    antenv.axon_hooks; degrade silently if anything is missing."""
    import types

    if "antenv.axon_hooks" in sys.modules:
        return
    try:
        from trn_agent_boot.trn_boot import _ntff_profile_via_ctypes

        hook = _ntff_profile_via_ctypes("/opt/axon/libaxon_pjrt.so")
        if hook is None:
            return
        mod = types.ModuleType("antenv.axon_hooks")
        mod.get_axon_ntff_profile_hook = lambda: hook
        mod.set_axon_ntff_profile_hook = lambda h: None
        sys.modules["antenv.axon_hooks"] = mod
        from concourse import bass_utils

        bass_utils.upload_artifacts = lambda tmpdir: f"local:{tmpdir}"
    except Exception:
        pass


_LAST_RESULTS = None


def kernel(x: np.ndarray, coeff: np.ndarray) -> np.ndarray:
    global _LAST_RESULTS
    from concourse import bass_utils

    _maybe_install_profile_shim()

    x = np.ascontiguousarray(np.asarray(x), dtype=np.float32)
    coeff = np.asarray(coeff)
    assert x.shape == (B_FULL, CIN, H, W), x.shape

    w_host, obias = _fold_coeff(coeff)
    btbl = np.zeros((128, NQ + 2), dtype=np.float32)
    for p in range(128):
        for q in range(NQ):
            m = 2 * q + (1 if p >= 64 else 0)
            btbl[p, q] = -float(m + 2)
    btbl[:, NQ] = 2.0
    btbl[:, NQ + 1] = C2

    nc = _build_bass()

    in_maps = []
    for i in range(N_CORES):
        in_maps.append({
            "x": np.ascontiguousarray(x[i * B_SHARD:(i + 1) * B_SHARD]),
            "w": w_host,
            "btbl": btbl,
            "obias": obias,
        })

    res = bass_utils.run_bass_kernel_spmd(
        nc, in_maps, core_ids=list(range(N_CORES)),
        trace=bool(os.environ.get("KAN_TRACE")),
    )
    _LAST_RESULTS = res

    out = np.concatenate([res.results[i]["out"] for i in range(N_CORES)], axis=0)
    return out.astype(np.float32, copy=False)



# revision 8
# speedup vs baseline: 1.3290x; 1.3290x over previous
"""Trainium2 Bass kernel for the ConvolutionalKAN problem.

Math: the KAN conv
    out[b,o,y,x] = sum_{j,kk,l,m} phi_m(11*inp[b,j,y+kk,x+l]) * coeff[o,j,kk,l,m]
with phi_m the degree-3 B-spline basis on uniform knots linspace(0,1,12).
Uniform knots -> phi_m(t) = N3(t-m) with N3 the cardinal cubic B-spline:
    6*N3 = a^3 - 4*b^3,  a = relu(2-u), b = relu(1-u) = relu(a-1), u = |t-(m+2)|
Weights fold to coeff/6 exactly, making this a VALID 3x3 conv over
64*8 = 512 input channels.

v2 design (vs the 217us baseline, which was elementwise-bound):
- The whole two-rail spline evaluation collapses to 2 Scalar ACTs
  (u = Abs(11x - (m+2)), a = Relu(2-u)) plus ONE custom DVE op
  KAN_CUBE: g = a^3 - 4*relu(a-1)^3 (exactly 8 ALU stages), registered
  at runtime in dve_ops.OPS. This removes ~350us of engine time.
- Basis tiles g are written in bf16 (halves SBUF + matmul streams at the
  same 1 col/cycle as f32r); weights cast to bf16.
- Matmuls run tap-major: for each (chunk q, tap) the stationary lhsT is
  loaded once and 8 interleaved psum chains (one per 8-row output group)
  stream against it, keeping the PE continuously busy so it ramps to the
  full 2.4 GHz p-state. Each group accumulates 36 matmuls (4 chunks x 9
  taps) into one PSUM bank; all 8 banks are in flight.
- Images are processed in row-halves (0..33 / 32..63) so the PE starts
  ~6us after kernel start instead of waiting for a full-image basis.

Sharding: data-parallel over batch, 2 images per core on 8 cores.
"""

import os
import sys

import numpy as np

for _p in ("/root/.axon_site/_ro/trn_rl_repo", "/opt/trn_rl_repo"):
    if os.path.isdir(_p) and _p not in sys.path:
        sys.path.append(_p)

B_FULL = 16
N_CORES = 8
B_SHARD = B_FULL // N_CORES
CIN = 64
COUT = 64
H = 64
W = 64
KS = 3
NB = 8
NS = 8
HO = H - KS + 1  # 62
WO = W - KS + 1  # 62
NQ = (CIN * NS) // 128  # 4 contraction tiles of 128
TAPS = KS * KS
N_STEPS = NQ * TAPS  # 36 accumulation steps per psum chain

# row halves: (first input row, n input rows); groups 0-3 read rows 0..33,
# groups 4-7 read rows 32..63
HALVES = [(0, 34), (32, 32)]
# output row groups: 8 groups of 8 rows (last has 6): group g = out rows
# 8g .. 8g+nr-1, reading input rows 8g .. 8g+nr+1
GROUPS = [(g, 8 if g < 7 else 6) for g in range(8)]

MM_DTYPE_ENV = os.environ.get("KAN_MM_DTYPE", "bfloat16")

_DVE_OP_CACHE = {}


def _get_kan_cube_op():
    """Register (once) and return the custom DVE op
    KAN_CUBE: out = in0^3 - 4*relu(in0-1)^3   (= 6*N3(u) for in0 = relu(2-u))
    """
    if "op" in _DVE_OP_CACHE:
        return _DVE_OP_CACHE["op"]
    from concourse import dve_ops
    from concourse.dve_spec import C2, One, Spec, Src0, lower, relu, sq
    from concourse.dve_uop import DveOpSpec

    name = "KAN_CUBE_V1"
    a = Src0
    b = relu(a - One)
    spec = Spec(
        body=sq(a) * a + sq(b) * b * C2,
        reference=lambda in0, in1, s0, s1, imm2: (
            in0**3 + np.maximum(in0 - 1.0, 0.0) ** 3 * imm2
        ).astype(np.float32),
    )
    existing = {op.name for op in dve_ops.OPS}
    if name not in existing:
        row = dve_ops._CUSTOM_DVE_ROW_BASE + len(dve_ops.OPS)
        shas = {}
        for ver in ("v3", "v4"):
            s = DveOpSpec(name=name, opcode=row, uops=lower(spec, ver=ver),
                          rd1_en=False)
            shas[ver] = s.sha(ver)
        op = dve_ops.DveOp(name, spec, subdim=False, uops_sha=shas)
        dve_ops.OPS.append(op)
        dve_ops._SUB_OPCODE_FOR_NAME[name] = row
    else:
        op = next(o for o in dve_ops.OPS if o.name == name)
    _DVE_OP_CACHE["op"] = op
    return op


def _fold_coeff(coeff: np.ndarray):
    """coeff [COUT, CIN, KS, KS, NB] -> W_host [512, 576] f32.

    Channels fed to the matmul are 6*phi_m(t), so the folded weights are
    coeff/6 in layout W_host[m*64 + j, (kk*3+l)*64 + o].
    """
    w = (coeff.astype(np.float64).transpose(4, 1, 2, 3, 0) / 6.0).reshape(
        NS * CIN, TAPS * COUT)
    return np.ascontiguousarray(w, dtype=np.float32)


def _build_bass():
    import concourse.bacc as bacc
    import concourse.mybir as mybir
    import concourse.tile as tile

    f32 = mybir.dt.float32
    mm_dt = getattr(mybir.dt, MM_DTYPE_ENV)
    AF = mybir.ActivationFunctionType
    kan_op = _get_kan_cube_op()

    nc = bacc.Bacc("TRN2", target_bir_lowering=False, debug=False,
                   num_devices=N_CORES)
    x_d = nc.dram_tensor("x", [B_SHARD, CIN, H, W], f32, kind="ExternalInput").ap()
    w_d = nc.dram_tensor("w", [NS * CIN, TAPS * COUT], f32, kind="ExternalInput").ap()
    b_d = nc.dram_tensor("btbl", [128, NQ + 1], f32, kind="ExternalInput").ap()
    out_d = nc.dram_tensor("out", [B_SHARD, COUT, HO, WO], f32,
                           kind="ExternalOutput").ap()

    with tile.TileContext(nc) as tc:
        from contextlib import ExitStack

        with ExitStack() as ctx:
            wpool = ctx.enter_context(tc.tile_pool(name="w", bufs=NQ))
            cpool = ctx.enter_context(tc.tile_pool(name="const", bufs=1))
            xpool = ctx.enter_context(tc.tile_pool(name="x", bufs=4))
            rpool = ctx.enter_context(tc.tile_pool(name="r", bufs=3))
            gpool = ctx.enter_context(tc.tile_pool(name="g", bufs=2 * NQ * 2))
            opool = ctx.enter_context(tc.tile_pool(name="o", bufs=4))
            ppool = ctx.enter_context(
                tc.tile_pool(name="ps", bufs=1, space="PSUM"))

            bt = cpool.tile([128, NQ + 1], f32)
            nc.sync.dma_start(bt[:], b_d[:])
            wts = []
            for q in range(NQ):
                wt = wpool.tile([128, TAPS * COUT], f32, tag="wstage")
                nc.sync.dma_start(wt[:], w_d[q * 128:(q + 1) * 128, :])
                if mm_dt != f32:
                    wr = wpool.tile([128, TAPS * COUT], mm_dt, tag="wr")
                    nc.vector.tensor_copy(wr[:], wt[:])
                    wts.append(wr)
                else:
                    wts.append(wt)

            for b in range(B_SHARD):
                # --- basis: per half, per chunk ---
                gts = {}  # (q, h) -> g tile [128, nrows*W] mm_dt
                for h, (y0, nin) in enumerate(HALVES):
                    npx = nin * W
                    xt = xpool.tile([128, npx], f32, tag="xt")
                    src = x_d[b, :, y0:y0 + nin, :]
                    xv = xt[:].rearrange("p (r c) -> p r c", c=W)
                    nc.gpsimd.dma_start(xv[0:64], src)
                    nc.gpsimd.dma_start(xv[64:128], src)
                    for q in range(NQ):
                        u = rpool.tile([128, npx], f32, tag="u")
                        a = rpool.tile([128, npx], f32, tag="a")
                        g = gpool.tile([128, npx], mm_dt, tag="g")
                        nc.scalar.activation(u[:], xt[:], AF.Abs,
                                             bias=bt[:, q:q + 1], scale=11.0)
                        nc.scalar.activation(a[:], u[:], AF.Relu,
                                             bias=bt[:, NQ:NQ + 1], scale=-1.0)
                        nc.vector._custom_dve(kan_op, out=g[:], in0=a[:],
                                              imm2=-4.0)
                        gts[(q, h)] = g

                gvs = {k: g[:].rearrange("p (r c) -> p r c", c=W)
                       for k, g in gts.items()}

                # --- matmuls: tap-major, 8 interleaved psum chains ---
                pss = [ppool.tile([64, 8, WO], f32, tag=f"ps{g}",
                                  name=f"ps{g}")
                       for g, _ in GROUPS]
                step = 0
                for q in range(NQ):
                    for kk in range(KS):
                        for l in range(KS):
                            lhsT = wts[q][:, (kk * KS + l) * COUT:
                                          (kk * KS + l + 1) * COUT]
                            for g, nr in GROUPS:
                                h = g // 4
                                y0 = HALVES[h][0]
                                r0 = 8 * g + kk - y0
                                rhs = gvs[(q, h)][:, r0:r0 + nr, l:l + WO]
                                nc.tensor.matmul(
                                    pss[g][:, :nr, :], lhsT, rhs,
                                    start=(step == 0),
                                    stop=(step == N_STEPS - 1),
                                )
                            step += 1

                # --- drain: psum -> sbuf -> dram ---
                for g, nr in GROUPS:
                    ot = opool.tile([64, 8, WO], f32, tag="ot")
                    nc.vector.tensor_copy(ot[:, :nr, :], pss[g][:, :nr, :])
                    nc.sync.dma_start(
                        out_d[b, :, 8 * g:8 * g + nr, :], ot[:, :nr, :])

    nc.compile()
    return nc


def _maybe_install_profile_shim():
    """Allow trace=True/BASS_TRACE under axon even though this image lacks
    antenv.axon_hooks; degrade silently if anything is missing."""
    import types

    if "antenv.axon_hooks" in sys.modules:
        return
    try:
        from trn_agent_boot.trn_boot import _ntff_profile_via_ctypes

        hook = _ntff_profile_via_ctypes("/opt/axon/libaxon_pjrt.so")
        if hook is None:
            return
        mod = types.ModuleType("antenv.axon_hooks")
        mod.get_axon_ntff_profile_hook = lambda: hook
        mod.set_axon_ntff_profile_hook = lambda h: None
        sys.modules["antenv.axon_hooks"] = mod
        from concourse import bass_utils

        bass_utils.upload_artifacts = lambda tmpdir: f"local:{tmpdir}"
    except Exception:
        pass


_LAST_RESULTS = None


def kernel(x: np.ndarray, coeff: np.ndarray) -> np.ndarray:
    global _LAST_RESULTS
    from concourse import bass_utils

    _maybe_install_profile_shim()

    x = np.ascontiguousarray(np.asarray(x), dtype=np.float32)
    coeff = np.asarray(coeff)
    assert x.shape == (B_FULL, CIN, H, W), x.shape

    w_host = _fold_coeff(coeff)
    btbl = np.zeros((128, NQ + 1), dtype=np.float32)
    for p in range(128):
        for q in range(NQ):
            m = 2 * q + (1 if p >= 64 else 0)
            btbl[p, q] = -float(m + 2)
    btbl[:, NQ] = 2.0

    nc = _build_bass()

    in_maps = []
    for i in range(N_CORES):
        in_maps.append({
            "x": np.ascontiguousarray(x[i * B_SHARD:(i + 1) * B_SHARD]),
            "w": w_host,
            "btbl": btbl,
        })

    res = bass_utils.run_bass_kernel_spmd(
        nc, in_maps, core_ids=list(range(N_CORES)),
        trace=bool(os.environ.get("KAN_TRACE")),
    )
    _LAST_RESULTS = res

    out = np.concatenate([res.results[i]["out"] for i in range(N_CORES)], axis=0)
    return out.astype(np.float32, copy=False)
